# revision 6
# baseline (speedup 1.0000x reference)
"""CAAN (cross-asset attention) Trainium2 kernel.

Reference computation (B=32, N=2048, D=256):
    q = x@Wq + bq;  k = x@Wk + bk;  v = x@Wv + bv
    beta = softmax(q @ k^T / sqrt(D), axis=-1)
    out  = (beta @ v) @ Ww + bw            # [B, N]

Algebraic restructuring used here:
    (beta @ v) @ Ww == beta @ (v @ Ww)     # associativity: kills the BxNxNxD einsum
    v @ Ww == x @ (Wv @ Ww) + bv.Ww       # fold v-projection into a vector u
    softmax without max-subtraction (logits are O(6)): P = exp(q k^T / 16)
    out[i] = sum_j P[ij] u0[j] / sum_j P[ij] + (bv.Ww + bw)
where u0 = x @ (Wv@Ww).  The additive constant commutes through the
softmax average and is applied on the host.

Device layout (per core, 4 batches):
    xT  [d, n] (host pre-transposed)  ->  qT,kT [e, n] via PE projections
    sT[j, i] = sum_e kT[e,j] qT[e,i]  (PE, PSUM)  ->  P^T = exp(sT) (ScalarE)
    [su; rs][i] = [u|1]^T @ P^T       (PE, 2-wide stationary)
Host: out = su/rs + (bv.Ww + bw), sharded data-parallel over B across 8 cores.
"""

import numpy as np

import concourse.bass as bass
import concourse.bacc as bacc
import concourse.tile as tile
from concourse import mybir
from concourse.bass_utils import run_bass_kernel_spmd

B, N, D = 32, 2048, 256
NCORES = 8
BPC = B // NCORES  # batches per core
P = 128            # partitions
DC = D // P        # contraction chunks (2)
FB = 512           # matmul free-dim block (one PSUM bank, fp32)
NB = N // FB       # free blocks per row (4)
NJ = N // P        # j tiles (16)

F32 = mybir.dt.float32

_CACHE = {}
LAST_EXEC_NS = None


def _build_program():
    nc = bacc.Bacc("TRN2")

    xt = nc.dram_tensor("xt", [BPC, D, N], F32, kind="ExternalInput")
    wq = nc.dram_tensor("wq", [D, D], F32, kind="ExternalInput")
    wk = nc.dram_tensor("wk", [D, D], F32, kind="ExternalInput")
    bq = nc.dram_tensor("bq", [P, DC], F32, kind="ExternalInput")
    bk = nc.dram_tensor("bk", [P, DC], F32, kind="ExternalInput")
    wv = nc.dram_tensor("wv", [P, DC], F32, kind="ExternalInput")
    sr = nc.dram_tensor("sr", [BPC, 2, N], F32, kind="ExternalOutput")

    with tile.TileContext(nc) as tc:
        with (
            tc.tile_pool(name="consts", bufs=1) as consts,
            tc.tile_pool(name="xtp", bufs=2) as xtp,
            tc.tile_pool(name="qtp", bufs=2) as qtp,
            tc.tile_pool(name="ktp", bufs=2) as ktp,
            tc.tile_pool(name="ptp", bufs=4) as ptp,
            tc.tile_pool(name="uop", bufs=2) as uop,
            tc.tile_pool(name="outp", bufs=2) as outp,
            tc.tile_pool(name="ps_proj", bufs=2, space="PSUM") as ps_proj,
            tc.tile_pool(name="ps_u", bufs=2, space="PSUM") as ps_u,
            tc.tile_pool(name="ps_s", bufs=2, space="PSUM") as ps_s,
            tc.tile_pool(name="ps_r", bufs=2, space="PSUM") as ps_r,
        ):
            wq_sb = consts.tile([P, DC, D], F32)
            wk_sb = consts.tile([P, DC, D], F32)
            bq_sb = consts.tile([P, DC], F32)
            bk_sb = consts.tile([P, DC], F32)
            wv_sb = consts.tile([P, DC], F32)
            for dc in range(DC):
                nc.sync.dma_start(out=wq_sb[:, dc, :], in_=wq[dc * P:(dc + 1) * P, :])
                nc.sync.dma_start(out=wk_sb[:, dc, :], in_=wk[dc * P:(dc + 1) * P, :])
            nc.sync.dma_start(out=bq_sb, in_=bq[:, :])
            nc.sync.dma_start(out=bk_sb, in_=bk[:, :])
            nc.sync.dma_start(out=wv_sb, in_=wv[:, :])

            for b in range(BPC):
                xt_sb = xtp.tile([P, DC, N], F32)
                for dc in range(DC):
                    nc.sync.dma_start(
                        out=xt_sb[:, dc, :], in_=xt[b, dc * P:(dc + 1) * P, :]
                    )

                qt_sb = qtp.tile([P, DC, N], F32)
                kt_sb = ktp.tile([P, DC, N], F32)

                # Projections: qT[e, n] = sum_d Wq[d, e] xT[d, n]  (+bias)
                for w_sb, b_sb, dst in (
                    (wq_sb, bq_sb, qt_sb),
                    (wk_sb, bk_sb, kt_sb),
                ):
                    for ec in range(DC):
                        for ib in range(NB):
                            ps = ps_proj.tile([P, FB], F32, tag="ps")
                            for dc in range(DC):
                                nc.tensor.matmul(
                                    ps,
                                    lhsT=w_sb[:, dc, ec * P:(ec + 1) * P],
                                    rhs=xt_sb[:, dc, ib * FB:(ib + 1) * FB],
                                    start=(dc == 0),
                                    stop=(dc == DC - 1),
                                )
                            nc.vector.tensor_scalar_add(
                                out=dst[:, ec, ib * FB:(ib + 1) * FB],
                                in0=ps,
                                scalar1=b_sb[:, ec:ec + 1],
                            )

                # u0[j] = sum_d xT[d, j] wv[d], partition-major; odd cols = 1.0
                uo_sb = uop.tile([P, 2 * NJ], F32)
                nc.vector.memset(uo_sb, 1.0)
                for jt in range(NJ):
                    psu = ps_u.tile([P, 1], F32, tag="psu")
                    for dc in range(DC):
                        nc.tensor.matmul(
                            psu,
                            lhsT=xt_sb[:, dc, jt * P:(jt + 1) * P],
                            rhs=wv_sb[:, dc:dc + 1],
                            start=(dc == 0),
                            stop=(dc == DC - 1),
                        )
                    nc.vector.tensor_copy(out=uo_sb[:, 2 * jt:2 * jt + 1], in_=psu)

                # Attention: for each i-block accumulate [su; rs] over j tiles
                out_sb = outp.tile([2, N], F32)
                for ib in range(NB):
                    red = ps_r.tile([2, FB], F32, tag="red")
                    for jt in range(NJ):
                        sT = ps_s.tile([P, FB], F32, tag="sT")
                        for ec in range(DC):
                            nc.tensor.matmul(
                                sT,
                                lhsT=kt_sb[:, ec, jt * P:(jt + 1) * P],
                                rhs=qt_sb[:, ec, ib * FB:(ib + 1) * FB],
                                start=(ec == 0),
                                stop=(ec == DC - 1),
                            )
                        pT = ptp.tile([P, FB], F32)
                        nc.scalar.activation(
                            out=pT, in_=sT, func=mybir.ActivationFunctionType.Exp
                        )
                        nc.tensor.matmul(
                            red,
                            lhsT=uo_sb[:, 2 * jt:2 * jt + 2],
                            rhs=pT,
                            start=(jt == 0),
                            stop=(jt == NJ - 1),
                        )
                    nc.vector.tensor_copy(
                        out=out_sb[:, ib * FB:(ib + 1) * FB], in_=red
                    )
                nc.sync.dma_start(out=sr[b, :, :], in_=out_sb)

    nc.compile()
    return nc


def kernel(x, Wq, bq, Wk, bk, Wv, bv, Ww, bw, trace=False):
    global LAST_EXEC_NS
    x = np.asarray(x, dtype=np.float32)
    Wq = np.asarray(Wq, dtype=np.float32)
    bq = np.asarray(bq, dtype=np.float32)
    Wk = np.asarray(Wk, dtype=np.float32)
    bk = np.asarray(bk, dtype=np.float32)
    Wv = np.asarray(Wv, dtype=np.float32)
    bv = np.asarray(bv, dtype=np.float32)
    Ww = np.asarray(Ww, dtype=np.float32)
    bw = np.asarray(bw, dtype=np.float32)

    scale = np.float32(1.0 / np.sqrt(D))
    wq_h = np.ascontiguousarray(Wq * scale)
    bq_h = np.ascontiguousarray((bq * scale).reshape(DC, P).T)
    wk_h = np.ascontiguousarray(Wk)
    bk_h = np.ascontiguousarray(bk.reshape(DC, P).T)
    u_w = (Wv @ Ww)[:, 0]                       # [D]
    wv_h = np.ascontiguousarray(u_w.reshape(DC, P).T)
    const_add = float(bv @ Ww[:, 0]) + float(bw[0])

    xt_all = np.ascontiguousarray(x.transpose(0, 2, 1))  # [B, D, N]

    if "nc" not in _CACHE:
        _CACHE["nc"] = _build_program()
    nc = _CACHE["nc"]

    in_maps = []
    for c in range(NCORES):
        in_maps.append({
            "xt": np.ascontiguousarray(xt_all[c * BPC:(c + 1) * BPC]),
            "wq": wq_h, "wk": wk_h,
            "bq": bq_h, "bk": bk_h, "wv": wv_h,
        })

    res = run_bass_kernel_spmd(nc, in_maps, core_ids=list(range(NCORES)), trace=trace)
    LAST_EXEC_NS = res.exec_time_ns

    out = np.empty((B, N), dtype=np.float32)
    for c in range(NCORES):
        sr = res.results[c]["sr"]  # [BPC, 2, N]
        su = sr[:, 0, :].astype(np.float64)
        rs = sr[:, 1, :].astype(np.float64)
        out[c * BPC:(c + 1) * BPC] = (su / rs + const_add).astype(np.float32)
    return out


# revision 18
# speedup vs baseline: 3.3596x; 3.3596x over previous
"""CAAN (cross-asset attention) Trainium2 kernel.

Reference computation (B=32, N=2048, D=256):
    q = x@Wq + bq;  k = x@Wk + bk;  v = x@Wv + bv
    beta = softmax(q @ k^T / sqrt(D), axis=-1)
    out  = (beta @ v) @ Ww + bw            # [B, N]

Algebraic restructuring used here:
    (beta @ v) @ Ww == beta @ (v @ Ww)     # associativity: kills the BxNxNxD einsum
    v @ Ww == x @ (Wv @ Ww) + bv.Ww       # fold v-projection into a vector u
    softmax without max-subtraction (logits are O(6)): P = exp(q k^T / 16)
    out[i] = sum_j P[ij] u0[j] / sum_j P[ij] + (bv.Ww + bw)
where u0 = x @ (Wv@Ww).  The additive constant commutes through the
softmax average and is applied on the host.

Device layout (per core, 4 batches):
    xT  [d, n] (host pre-transposed)  ->  qT,kT [e, n] via PE projections
    sT[j, i] = sum_e kT[e,j] qT[e,i]  (PE, PSUM)  ->  P^T = exp(sT) (ScalarE)
    [su; rs][i] = [u|1]^T @ P^T       (PE, 2-wide stationary)
Host: out = su/rs + (bv.Ww + bw), sharded data-parallel over B across 8 cores.
"""

import numpy as np

import concourse.bass as bass
import concourse.bacc as bacc
import concourse.tile as tile
from concourse import mybir
from concourse.bass_utils import run_bass_kernel_spmd

B, N, D = 32, 2048, 256
NCORES = 8
BPC = B // NCORES  # batches per core
P = 128            # partitions
DC = D // P        # contraction chunks (2)
FB = 512           # matmul free-dim block (one PSUM bank, fp32)
NB = N // FB       # free blocks per row (4)
NJ = N // P        # j tiles (16)

F32 = mybir.dt.float32
F32R = mybir.dt.float32r

_CACHE = {}
LAST_EXEC_NS = None


def _r(ap):
    """View an fp32 AP as float32r for single-pass PE matmuls."""
    return ap.bitcast(F32R)


def _build_program():
    nc = bacc.Bacc("TRN2")

    xt = nc.dram_tensor("xt", [BPC, D, N], F32, kind="ExternalInput")
    wq = nc.dram_tensor("wq", [D, D], F32, kind="ExternalInput")
    wk = nc.dram_tensor("wk", [D, D], F32, kind="ExternalInput")
    bq = nc.dram_tensor("bq", [P, DC], F32, kind="ExternalInput")
    bk = nc.dram_tensor("bk", [P, DC], F32, kind="ExternalInput")
    wv = nc.dram_tensor("wv", [P, DC, 2], F32, kind="ExternalInput")
    ones = nc.dram_tensor("ones", [P, 2 * NJ], F32, kind="ExternalInput")
    sr = nc.dram_tensor("sr", [BPC, 2, N], F32, kind="ExternalOutput")

    with tile.TileContext(nc) as tc:
        with (
            tc.tile_pool(name="consts", bufs=1) as consts,
            tc.tile_pool(name="xtp", bufs=2) as xtp,
            tc.tile_pool(name="qtp", bufs=2) as qtp,
            tc.tile_pool(name="ktp", bufs=2) as ktp,
            tc.tile_pool(name="ptp", bufs=4) as ptp,
            tc.tile_pool(name="uop", bufs=2) as uop,
            tc.tile_pool(name="outp", bufs=2) as outp,
            tc.tile_pool(name="ps_proj", bufs=2, space="PSUM") as ps_proj,
            tc.tile_pool(name="ps_s", bufs=2, space="PSUM") as ps_s,
            tc.tile_pool(name="ps_r", bufs=2, space="PSUM") as ps_r,
        ):
            wq_sb = consts.tile([P, DC, D], F32R)
            wk_sb = consts.tile([P, DC, D], F32R)
            bq_sb = consts.tile([P, DC], F32)
            bk_sb = consts.tile([P, DC], F32)
            wv_sb = consts.tile([P, DC, 2], F32R)
            for dc in range(DC):
                nc.sync.dma_start(out=wq_sb[:, dc, :], in_=_r(wq[dc * P:(dc + 1) * P, :]))
                nc.sync.dma_start(out=wk_sb[:, dc, :], in_=_r(wk[dc * P:(dc + 1) * P, :]))
            nc.sync.dma_start(out=bq_sb, in_=bq[:, :])
            nc.sync.dma_start(out=bk_sb, in_=bk[:, :])
            nc.sync.dma_start(out=wv_sb, in_=_r(wv[:, :, :]))

            for b in range(BPC):
                xt_sb = xtp.tile([P, DC, N], F32R)
                for dc in range(DC):
                    nc.sync.dma_start(
                        out=xt_sb[:, dc, :], in_=_r(xt[b, dc * P:(dc + 1) * P, :])
                    )

                qt_sb = qtp.tile([P, DC, N], F32R)
                kt_sb = ktp.tile([P, DC, N], F32R)

                # Projections: qT[e, n] = sum_d Wq[d, e] xT[d, n]  (+bias)
                for w_sb, b_sb, dst in (
                    (wq_sb, bq_sb, qt_sb),
                    (wk_sb, bk_sb, kt_sb),
                ):
                    for ec in range(DC):
                        for ib in range(NB):
                            ps = ps_proj.tile([P, FB], F32, tag="ps")
                            for dc in range(DC):
                                nc.tensor.matmul(
                                    ps,
                                    lhsT=w_sb[:, dc, ec * P:(ec + 1) * P],
                                    rhs=xt_sb[:, dc, ib * FB:(ib + 1) * FB],
                                    start=(dc == 0),
                                    stop=(dc == DC - 1),
                                )
                            nc.vector.tensor_scalar_add(
                                out=dst[:, ec, ib * FB:(ib + 1) * FB],
                                in0=ps,
                                scalar1=b_sb[:, ec:ec + 1],
                            )

                # u0[j] = sum_d xT[d, j] wv[d], partition-major; odd cols = 1.0
                uo_sb = uop.tile([P, 2 * NJ], F32R)
                nc.sync.dma_start(out=uo_sb, in_=_r(ones[:, :]))
                for jt in range(NJ):
                    psu = ps_proj.tile([P, 2], F32, tag="ps")
                    for dc in range(DC):
                        nc.tensor.matmul(
                            psu,
                            lhsT=xt_sb[:, dc, jt * P:(jt + 1) * P],
                            rhs=wv_sb[:, dc, :],
                            start=(dc == 0),
                            stop=(dc == DC - 1),
                        )
                    nc.vector.tensor_copy(out=uo_sb[:, 2 * jt:2 * jt + 1], in_=psu[:, 0:1])

                # Attention: for each i-block accumulate [su; rs] over j tiles.
                # sT/exp batched over 1024-wide blocks (2 PSUM banks) to halve
                # ScalarE op count; matmuls stay 512-wide (fp32 PSUM bank cap).
                out_sb = outp.tile([2, N], F32)
                for ib2 in range(NB // 2):
                    reds = [
                        ps_r.tile([2, FB], F32, tag="red", name=f"red{h}")
                        for h in range(2)
                    ]
                    for jt in range(NJ):
                        sT = ps_s.tile([P, 2 * FB], F32, tag="sT")
                        for half in range(2):
                            ib = ib2 * 2 + half
                            for ec in range(DC):
                                nc.tensor.matmul(
                                    sT[:, half * FB:(half + 1) * FB],
                                    lhsT=kt_sb[:, ec, jt * P:(jt + 1) * P],
                                    rhs=qt_sb[:, ec, ib * FB:(ib + 1) * FB],
                                    start=(ec == 0),
                                    stop=(ec == DC - 1),
                                )
                        pT = ptp.tile([P, 2 * FB], F32R)
                        nc.scalar.activation(
                            out=pT, in_=sT, func=mybir.ActivationFunctionType.Exp
                        )
                        for half in range(2):
                            nc.tensor.matmul(
                                reds[half],
                                lhsT=uo_sb[:, 2 * jt:2 * jt + 2],
                                rhs=pT[:, half * FB:(half + 1) * FB],
                                start=(jt == 0),
                                stop=(jt == NJ - 1),
                            )
                    for half in range(2):
                        ib = ib2 * 2 + half
                        nc.vector.tensor_copy(
                            out=out_sb[:, ib * FB:(ib + 1) * FB], in_=reds[half]
                        )
                nc.sync.dma_start(out=sr[b, :, :], in_=out_sb)

    nc.compile()
    return nc


def kernel(x, Wq, bq, Wk, bk, Wv, bv, Ww, bw, trace=False):
    global LAST_EXEC_NS
    x = np.asarray(x, dtype=np.float32)
    Wq = np.asarray(Wq, dtype=np.float32)
    bq = np.asarray(bq, dtype=np.float32)
    Wk = np.asarray(Wk, dtype=np.float32)
    bk = np.asarray(bk, dtype=np.float32)
    Wv = np.asarray(Wv, dtype=np.float32)
    bv = np.asarray(bv, dtype=np.float32)
    Ww = np.asarray(Ww, dtype=np.float32)
    bw = np.asarray(bw, dtype=np.float32)

    scale = np.float32(1.0 / np.sqrt(D))
    wq_h = np.ascontiguousarray(Wq * scale)
    bq_h = np.ascontiguousarray((bq * scale).reshape(DC, P).T)
    wk_h = np.ascontiguousarray(Wk)
    bk_h = np.ascontiguousarray(bk.reshape(DC, P).T)
    u_w = (Wv @ Ww)[:, 0]                       # [D]
    wv_h = np.zeros((P, DC, 2), dtype=np.float32)
    wv_h[:, :, 0] = u_w.reshape(DC, P).T
    const_add = float(bv @ Ww[:, 0]) + float(bw[0])

    xt_all = np.ascontiguousarray(x.transpose(0, 2, 1))  # [B, D, N]

    if "nc" not in _CACHE:
        _CACHE["nc"] = _build_program()
    nc = _CACHE["nc"]

    in_maps = []
    for c in range(NCORES):
        in_maps.append({
            "xt": np.ascontiguousarray(xt_all[c * BPC:(c + 1) * BPC]),
            "wq": wq_h, "wk": wk_h,
            "bq": bq_h, "bk": bk_h, "wv": wv_h,
            "ones": np.ones((P, 2 * NJ), dtype=np.float32),
        })

    res = run_bass_kernel_spmd(nc, in_maps, core_ids=list(range(NCORES)), trace=trace)
    LAST_EXEC_NS = res.exec_time_ns

    out = np.empty((B, N), dtype=np.float32)
    for c in range(NCORES):
        sr = res.results[c]["sr"]  # [BPC, 2, N]
        su = sr[:, 0, :].astype(np.float64)
        rs = sr[:, 1, :].astype(np.float64)
        out[c * BPC:(c + 1) * BPC] = (su / rs + const_add).astype(np.float32)
    return out


# revision 20
# speedup vs baseline: 3.4067x; 1.0140x over previous
"""CAAN (cross-asset attention) Trainium2 kernel.

Reference computation (B=32, N=2048, D=256):
    q = x@Wq + bq;  k = x@Wk + bk;  v = x@Wv + bv
    beta = softmax(q @ k^T / sqrt(D), axis=-1)
    out  = (beta @ v) @ Ww + bw            # [B, N]

Algebraic restructuring used here:
    (beta @ v) @ Ww == beta @ (v @ Ww)     # associativity: kills the BxNxNxD einsum
    v @ Ww == x @ (Wv @ Ww) + bv.Ww       # fold v-projection into a vector u
    softmax without max-subtraction (logits are O(6)): P = exp(q k^T / 16)
    out[i] = sum_j P[ij] u0[j] / sum_j P[ij] + (bv.Ww + bw)
where u0 = x @ (Wv@Ww).  The additive constant commutes through the
softmax average and is applied on the host.

Device layout (per core, 4 batches):
    xT  [d, n] (host pre-transposed)  ->  qT,kT [e, n] via PE projections
    sT[j, i] = sum_e kT[e,j] qT[e,i]  (PE, PSUM)  ->  P^T = exp(sT) (ScalarE)
    [su; rs][i] = [u|1]^T @ P^T       (PE, 2-wide stationary)
Host: out = su/rs + (bv.Ww + bw), sharded data-parallel over B across 8 cores.
"""

import numpy as np

import concourse.bass as bass
import concourse.bacc as bacc
import concourse.tile as tile
from concourse import mybir
from concourse.bass_utils import run_bass_kernel_spmd

B, N, D = 32, 2048, 256
NCORES = 8
BPC = B // NCORES  # batches per core
P = 128            # partitions
DC = D // P        # contraction chunks (2)
FB = 512           # matmul free-dim block (one PSUM bank, fp32)
NB = N // FB       # free blocks per row (4)
NJ = N // P        # j tiles (16)

F32 = mybir.dt.float32
F32R = mybir.dt.float32r

_CACHE = {}
LAST_EXEC_NS = None


def _r(ap):
    """View an fp32 AP as float32r for single-pass PE matmuls."""
    return ap.bitcast(F32R)


def _build_program():
    nc = bacc.Bacc("TRN2")

    xt = nc.dram_tensor("xt", [BPC, D, N], F32, kind="ExternalInput")
    wq = nc.dram_tensor("wq", [D, D], F32, kind="ExternalInput")
    wk = nc.dram_tensor("wk", [D, D], F32, kind="ExternalInput")
    bq = nc.dram_tensor("bq", [P, DC], F32, kind="ExternalInput")
    bk = nc.dram_tensor("bk", [P, DC], F32, kind="ExternalInput")
    wv = nc.dram_tensor("wv", [P, DC, 2], F32, kind="ExternalInput")
    ones = nc.dram_tensor("ones", [P, 2 * NJ], F32, kind="ExternalInput")
    sr = nc.dram_tensor("sr", [BPC, 2, N], F32, kind="ExternalOutput")

    with tile.TileContext(nc) as tc:
        with (
            tc.tile_pool(name="consts", bufs=1) as consts,
            tc.tile_pool(name="xtp", bufs=2) as xtp,
            tc.tile_pool(name="qtp", bufs=2) as qtp,
            tc.tile_pool(name="ktp", bufs=2) as ktp,
            tc.tile_pool(name="ptp", bufs=6) as ptp,
            tc.tile_pool(name="uop", bufs=2) as uop,
            tc.tile_pool(name="outp", bufs=2) as outp,
            tc.tile_pool(name="ps_proj", bufs=2, space="PSUM") as ps_proj,
            tc.tile_pool(name="ps_s", bufs=2, space="PSUM") as ps_s,
            tc.tile_pool(name="ps_r", bufs=2, space="PSUM") as ps_r,
        ):
            wq_sb = consts.tile([P, DC, D], F32R)
            wk_sb = consts.tile([P, DC, D], F32R)
            bq_sb = consts.tile([P, DC], F32)
            bk_sb = consts.tile([P, DC], F32)
            wv_sb = consts.tile([P, DC, 2], F32R)
            for dc in range(DC):
                nc.sync.dma_start(out=wq_sb[:, dc, :], in_=_r(wq[dc * P:(dc + 1) * P, :]))
                nc.sync.dma_start(out=wk_sb[:, dc, :], in_=_r(wk[dc * P:(dc + 1) * P, :]))
            nc.sync.dma_start(out=bq_sb, in_=bq[:, :])
            nc.sync.dma_start(out=bk_sb, in_=bk[:, :])
            nc.sync.dma_start(out=wv_sb, in_=_r(wv[:, :, :]))

            for b in range(BPC):
                xt_sb = xtp.tile([P, DC, N], F32R)
                for dc in range(DC):
                    # Per-block DMAs (256 KB) on the scalar HWDGE ring: the
                    # first projection matmuls only wait for their own block,
                    # cutting the kernel-entry PE idle gap.
                    for ib in range(NB):
                        nc.scalar.dma_start(
                            out=xt_sb[:, dc, ib * FB:(ib + 1) * FB],
                            in_=_r(xt[b, dc * P:(dc + 1) * P, ib * FB:(ib + 1) * FB]),
                        )

                qt_sb = qtp.tile([P, DC, N], F32R)
                kt_sb = ktp.tile([P, DC, N], F32R)

                # Projections: qT[e, n] = sum_d Wq[d, e] xT[d, n]  (+bias)
                for w_sb, b_sb, dst in (
                    (wq_sb, bq_sb, qt_sb),
                    (wk_sb, bk_sb, kt_sb),
                ):
                    for ec in range(DC):
                        for ib in range(NB):
                            ps = ps_proj.tile([P, FB], F32, tag="ps")
                            for dc in range(DC):
                                nc.tensor.matmul(
                                    ps,
                                    lhsT=w_sb[:, dc, ec * P:(ec + 1) * P],
                                    rhs=xt_sb[:, dc, ib * FB:(ib + 1) * FB],
                                    start=(dc == 0),
                                    stop=(dc == DC - 1),
                                )
                            nc.vector.tensor_scalar_add(
                                out=dst[:, ec, ib * FB:(ib + 1) * FB],
                                in0=ps,
                                scalar1=b_sb[:, ec:ec + 1],
                            )

                # u0[j] = sum_d xT[d, j] wv[d], partition-major; odd cols = 1.0
                uo_sb = uop.tile([P, 2 * NJ], F32R)
                nc.sync.dma_start(out=uo_sb, in_=_r(ones[:, :]))
                for jt in range(NJ):
                    psu = ps_proj.tile([P, 2], F32, tag="ps")
                    for dc in range(DC):
                        nc.tensor.matmul(
                            psu,
                            lhsT=xt_sb[:, dc, jt * P:(jt + 1) * P],
                            rhs=wv_sb[:, dc, :],
                            start=(dc == 0),
                            stop=(dc == DC - 1),
                        )
                    nc.vector.tensor_copy(out=uo_sb[:, 2 * jt:2 * jt + 1], in_=psu[:, 0:1])

                # Attention: for each i-block accumulate [su; rs] over j tiles.
                # sT/exp batched over 1024-wide blocks (2 PSUM banks) to halve
                # ScalarE op count; matmuls stay 512-wide (fp32 PSUM bank cap).
                out_sb = outp.tile([2, N], F32)
                for ib2 in range(NB // 2):
                    reds = [
                        ps_r.tile([2, FB], F32, tag="red", name=f"red{h}")
                        for h in range(2)
                    ]
                    for jt in range(NJ):
                        sT = ps_s.tile([P, 2 * FB], F32, tag="sT")
                        for half in range(2):
                            ib = ib2 * 2 + half
                            for ec in range(DC):
                                nc.tensor.matmul(
                                    sT[:, half * FB:(half + 1) * FB],
                                    lhsT=kt_sb[:, ec, jt * P:(jt + 1) * P],
                                    rhs=qt_sb[:, ec, ib * FB:(ib + 1) * FB],
                                    start=(ec == 0),
                                    stop=(ec == DC - 1),
                                )
                        pT = ptp.tile([P, 2 * FB], F32R)
                        nc.scalar.activation(
                            out=pT, in_=sT, func=mybir.ActivationFunctionType.Exp
                        )
                        for half in range(2):
                            nc.tensor.matmul(
                                reds[half],
                                lhsT=uo_sb[:, 2 * jt:2 * jt + 2],
                                rhs=pT[:, half * FB:(half + 1) * FB],
                                start=(jt == 0),
                                stop=(jt == NJ - 1),
                            )
                    for half in range(2):
                        ib = ib2 * 2 + half
                        nc.vector.tensor_copy(
                            out=out_sb[:, ib * FB:(ib + 1) * FB], in_=reds[half]
                        )
                nc.sync.dma_start(out=sr[b, :, :], in_=out_sb)

    nc.compile()
    return nc


def kernel(x, Wq, bq, Wk, bk, Wv, bv, Ww, bw, trace=False):
    global LAST_EXEC_NS
    x = np.asarray(x, dtype=np.float32)
    Wq = np.asarray(Wq, dtype=np.float32)
    bq = np.asarray(bq, dtype=np.float32)
    Wk = np.asarray(Wk, dtype=np.float32)
    bk = np.asarray(bk, dtype=np.float32)
    Wv = np.asarray(Wv, dtype=np.float32)
    bv = np.asarray(bv, dtype=np.float32)
    Ww = np.asarray(Ww, dtype=np.float32)
    bw = np.asarray(bw, dtype=np.float32)

    scale = np.float32(1.0 / np.sqrt(D))
    wq_h = np.ascontiguousarray(Wq * scale)
    bq_h = np.ascontiguousarray((bq * scale).reshape(DC, P).T)
    wk_h = np.ascontiguousarray(Wk)
    bk_h = np.ascontiguousarray(bk.reshape(DC, P).T)
    u_w = (Wv @ Ww)[:, 0]                       # [D]
    wv_h = np.zeros((P, DC, 2), dtype=np.float32)
    wv_h[:, :, 0] = u_w.reshape(DC, P).T
    const_add = float(bv @ Ww[:, 0]) + float(bw[0])

    xt_all = np.ascontiguousarray(x.transpose(0, 2, 1))  # [B, D, N]

    if "nc" not in _CACHE:
        _CACHE["nc"] = _build_program()
    nc = _CACHE["nc"]

    in_maps = []
    for c in range(NCORES):
        in_maps.append({
            "xt": np.ascontiguousarray(xt_all[c * BPC:(c + 1) * BPC]),
            "wq": wq_h, "wk": wk_h,
            "bq": bq_h, "bk": bk_h, "wv": wv_h,
            "ones": np.ones((P, 2 * NJ), dtype=np.float32),
        })

    res = run_bass_kernel_spmd(nc, in_maps, core_ids=list(range(NCORES)), trace=trace)
    LAST_EXEC_NS = res.exec_time_ns

    out = np.empty((B, N), dtype=np.float32)
    for c in range(NCORES):
        sr = res.results[c]["sr"]  # [BPC, 2, N]
        su = sr[:, 0, :].astype(np.float64)
        rs = sr[:, 1, :].astype(np.float64)
        out[c * BPC:(c + 1) * BPC] = (su / rs + const_add).astype(np.float32)
    return out
